# revision 28
# baseline (speedup 1.0000x reference)
"""Distributed Trainium2 (Bass/Tile) kernel for a causal RoPE attention block.

Reference computation (B=2, S=2048, D=1024, H=16, HD=64):
    qkv = (x @ W_in).reshape(B,S,H,3*HD); q,k,v = split(qkv)
    q,k = rope(q,pos), rope(k,pos); q /= sqrt(HD)
    scores = q @ k^T  (causal masked); attn = softmax(scores)
    out = (attn @ v).reshape(B,S,D) @ W_out

Sharding (8 cores): core c owns batch b=c//4 and heads 4*(c%4)..4*(c%4)+3.
QKV projection is column-parallel and attention fully local per head. The
per-head context (bf16) is exchanged with two 8-core AllToAlls (one per
head-pair) so every core ends up with the full 16-head context for a
256-row sequence slice of BOTH batches; the out-projection runs locally
against the full W_out and the output shards concatenate on the host.

Performance structure (PE util-throttle on TRN2 caps sustained matmul
throughput, so minimize PE cycles and keep every engine pipelined):
  - The two heads of a pair occupy PE row-groups 0-63 / 64-127 in the
    K=64 scores matmuls -> hardware runs them concurrently.
  - scores -> exp(ACT) -> AV is software-pipelined with a 1-step stagger.
  - softmax denominators use the fast custom-DVE reciprocal.
  - pair-0's AllToAll overlaps pair-1's attention; the out-projection
    accumulates pair-0 c-tiles first so they overlap the second AllToAll.
  - pair-1 Q/K projections and V-projection tile groups are emitted inside
    pair-0's attention stream to fill PE bubbles.

All matmuls run in bf16 with f32 PSUM accumulation. Softmax skips the
max-subtraction (scores are O(1) here) so exp(S) can accumulate straight
into PSUM via an appended ones-column on V that yields the row sums.
"""

import os
import sys
import numpy as np

for _p in ("/opt/trn_rl_repo", "/root/.axon_site/_ro/trn_rl_repo"):
    if _p not in sys.path and os.path.isdir(_p):
        sys.path.append(_p)

import ml_dtypes
from contextlib import ExitStack

import concourse.bass as bass
import concourse.mybir as mybir
import concourse.tile as tile
from concourse import bacc
from concourse.bass_utils import run_bass_kernel_spmd

F32 = mybir.dt.float32
BF16 = mybir.dt.bfloat16
BF = ml_dtypes.bfloat16

B, S, D, H, HD = 2, 2048, 1024, 16, 64
NCORES = 8
HPC = H // 4   # heads per core = 4
CW = HPC * HD  # per-core qkv slice width = 256
KT = 128       # k tile (partition dim of S^T tiles)
QB = 512       # q block (free dim / PSUM bank)
NKT = S // KT  # 16
NQB = S // QB  # 4
NDT = D // 128 # 8 contraction tiles
SC = S // 4    # per-core output sequence slice = 512
SH = S // NCORES  # 256 a2a chunk width

TRACE = False
SIM = False
FASTRCP = True
LAST = {}

_cache = {}


def _build(schedule, n_partial):
    """schedule[(kt,qb)] in {'full','skip'} or int partial-mask index."""
    nc = bacc.Bacc(
        "TRN2", target_bir_lowering=False, debug=False, num_devices=NCORES
    )

    xT = nc.dram_tensor("xT", [D, S], BF16, kind="ExternalInput")
    wq = nc.dram_tensor("wq", [D, CW], BF16, kind="ExternalInput")
    wk = nc.dram_tensor("wk", [D, CW], BF16, kind="ExternalInput")
    wv = nc.dram_tensor("wv", [D, CW], BF16, kind="ExternalInput")
    wo = nc.dram_tensor("wo", [D, D], BF16, kind="ExternalInput")
    tab = {}
    for t in ("cq", "sq", "ck", "sk"):
        tab[t] = nc.dram_tensor(t, [128, S], BF16, kind="ExternalInput")
    if n_partial:
        m01 = nc.dram_tensor("m01", [n_partial, KT, QB], BF16, kind="ExternalInput")
    out_e = nc.dram_tensor("out", [D, SC], F32, kind="ExternalOutput")

    with tile.TileContext(nc) as tc, ExitStack() as ctx:
        cst = ctx.enter_context(tc.tile_pool(name="cst", bufs=1))
        dram = ctx.enter_context(tc.tile_pool(name="dram", bufs=1, space="DRAM"))
        qraw_p = ctx.enter_context(tc.tile_pool(name="qraw", bufs=2))
        qswp_p = ctx.enter_context(tc.tile_pool(name="qswp", bufs=2))
        rtmp_p = ctx.enter_context(tc.tile_pool(name="rtmp", bufs=4))
        e_p = ctx.enter_context(tc.tile_pool(name="e", bufs=4))
        ctmp_p = ctx.enter_context(tc.tile_pool(name="ctmp", bufs=2))
        rcp_p = ctx.enter_context(tc.tile_pool(name="rcp", bufs=2))
        rb_p = ctx.enter_context(tc.tile_pool(name="rb", bufs=2))
        oT_p = ctx.enter_context(tc.tile_pool(name="oT", bufs=2))
        # PSUM: sps2 holds a head-PAIR of score tiles [128, 2*QB] (2 banks
        # each) so one ACT instruction exps both heads; Q/K/V projections
        # borrow sps2 slots (subranges) so the static pools fit 8 banks.
        sps2_p = ctx.enter_context(tc.tile_pool(name="sps2", bufs=2, space="PSUM"))
        mm_p = ctx.enter_context(tc.tile_pool(name="mm", bufs=2, space="PSUM"))
        cx_p = ctx.enter_context(tc.tile_pool(name="cx", bufs=2, space="PSUM"))

        # ---------------- input DMA (interleaved so K/Q proj start ASAP) ---
        xts, wqs, wks, wvs = [], [], [], []
        for d in range(NDT):
            t = cst.tile([128, S], BF16, tag=f"xT{d}", name=f"xT{d}")
            nc.sync.dma_start(t[:], xT.ap()[d * 128:(d + 1) * 128, :])
            xts.append(t)
            t = cst.tile([128, CW], BF16, tag=f"wk{d}", name=f"wk{d}")
            nc.sync.dma_start(t[:], wk.ap()[d * 128:(d + 1) * 128, :])
            wks.append(t)
            t = cst.tile([128, CW], BF16, tag=f"wq{d}", name=f"wq{d}")
            nc.sync.dma_start(t[:], wq.ap()[d * 128:(d + 1) * 128, :])
            wqs.append(t)
        tabs = {}
        for tn in ("ck", "sk"):
            t = cst.tile([128, S], BF16, tag=tn, name=f"tab_{tn}")
            nc.sync.dma_start(t[:], tab[tn].ap()[:, :])
            tabs[tn] = t
        # masks for q-block 0 (all four of its kt tiles are diagonal) must
        # arrive before the first attention steps
        mts = [None] * n_partial
        def load_mask(i):
            t = cst.tile([KT, QB], BF16, tag=f"m{i}", name=f"m{i}")
            nc.sync.dma_start(t[:], m01.ap()[i])
            mts[i] = t
        for i in range(min(4, n_partial)):
            load_mask(i)
        for d in range(NDT):
            t = cst.tile([128, CW], BF16, tag=f"wv{d}", name=f"wv{d}")
            nc.sync.dma_start(t[:], wv.ap()[d * 128:(d + 1) * 128, :])
            wvs.append(t)
        for tn in ("cq", "sq"):
            t = cst.tile([128, S], BF16, tag=tn, name=f"tab_{tn}")
            nc.sync.dma_start(t[:], tab[tn].ap()[:, :])
            tabs[tn] = t
        for i in range(4, n_partial):
            load_mask(i)
        wos = []
        for j in range(NDT):
            t = cst.tile([128, D], BF16, tag=f"wo{j}", name=f"wo{j}")
            nc.sync.dma_start(t[:], wo.ap()[j * 128:(j + 1) * 128, :])
            wos.append(t)

        # ---------------- Q/K projection + RoPE for one head-pair ---------
        qrot, krot = [], []
        for i in range(2):
            qrot.append(cst.tile([128, S], BF16, tag=f"qr{i}", name=f"qr{i}"))
            krot.append(cst.tile([128, S], BF16, tag=f"kr{i}", name=f"kr{i}"))

        def emit_qkproj(which, et):
            ws = wqs if which == "q" else wks
            ctab = tabs["cq" if which == "q" else "ck"]
            stab = tabs["sq" if which == "q" else "sk"]
            rots = qrot if which == "q" else krot
            raw = qraw_p.tile([128, S], BF16, tag="qraw", name=f"raw_{which}{et}")
            # d-outer with two sb blocks per sps2 slot: the first matmuls
            # only need xT tile 0, so the PE starts while the rest of the
            # input DMA stream is still arriving.
            for sp in range(2):
                ps = sps2_p.tile(
                    [128, 2 * QB], F32, tag="sps2", name=f"pj_{which}{et}{sp}"
                )
                for d in range(NDT):
                    for half in range(2):
                        sb = 2 * sp + half
                        nc.tensor.matmul(
                            ps[:, half * QB:(half + 1) * QB],
                            ws[d][:, et * 128:(et + 1) * 128],
                            xts[d][:, sb * QB:(sb + 1) * QB],
                            start=(d == 0), stop=(d == NDT - 1),
                        )
                nc.vector.tensor_copy(
                    raw[:, sp * 2 * QB:(sp + 1) * 2 * QB], ps[:]
                )
            # rotate-half partner: swap 32-row halves within each 64-row head
            swp = qswp_p.tile([128, S], BF16, tag="qswp", name=f"swp_{which}{et}")
            for g in range(4):
                src = (g ^ 1) * 32
                nc.sync.dma_start(
                    swp[g * 32:(g + 1) * 32, :], raw[src:src + 32, :]
                )
            # rot = raw*C + swp*Ssig   (C/Ssig fold the q scaling by 1/8)
            t1 = rtmp_p.tile([128, S], BF16, tag="rtmp", name=f"t1{which}{et}")
            t2 = rtmp_p.tile([128, S], BF16, tag="rtmp", name=f"t2{which}{et}")
            nc.vector.tensor_mul(t1[:], raw[:], ctab[:])
            nc.gpsimd.tensor_mul(t2[:], swp[:], stab[:])
            nc.vector.tensor_add(rots[et][:], t1[:], t2[:])

        # ---------------- V projection (natural layout + ones column) ------
        vplus = [None] * NKT

        def emit_vproj(st):
            vp = cst.tile([128, HPC * 65], BF16, tag=f"vp{st}", name=f"vp{st}")
            nc.vector.memset(vp[:], 1.0)
            vmm = sps2_p.tile([128, 2 * QB], F32, tag="sps2", name=f"vps{st}")
            vps = vmm[:, 0:CW]
            for d in range(NDT):
                nc.tensor.matmul(
                    vps, xts[d][:, st * 128:(st + 1) * 128], wvs[d][:],
                    start=(d == 0), stop=(d == NDT - 1),
                )
            for hl in range(HPC):
                nc.vector.tensor_copy(
                    vp[:, 65 * hl:65 * hl + 64], vps[:, 64 * hl:64 * hl + 64]
                )
            vplus[st] = vp

        # ---------------- attention ----------------
        # ctx2[i]: [128, S] bf16 — context^T for heads (2i, 2i+1).
        ctx2 = [
            cst.tile([128, S], BF16, tag=f"cx{i}", name=f"ctx2_{i}")
            for i in range(2)
        ]

        def emit_attention(i, qb):
            """Both heads of pair i for q block qb. The two heads' K=64
            scores occupy PE row-groups 0-63/64-127 and target the two
            banks of one sps2 tile -> concurrent execution + ONE exp
            activation for the pair. AV runs two kt behind scores so the
            exp+mask chain is fully hidden."""
            qsl = slice(qb * QB, (qb + 1) * QB)
            kts = [kt for kt in range(NKT) if schedule[(kt, qb)] != "skip"]
            cps = [
                cx_p.tile([65, QB], F32, tag="cx", name=f"cps{i}{qb}{h}")
                for h in range(2)
            ]
            es = []

            def emit_av(idx, is_last):
                kt = kts[idx]
                for h in range(2):
                    hl = 2 * i + h
                    nc.tensor.matmul(
                        cps[h][:], vplus[kt][:, 65 * hl:65 * hl + 65],
                        es[idx][:, h * QB:(h + 1) * QB],
                        start=(idx == 0), stop=is_last,
                    )

            stag = 2
            for n, kt in enumerate(kts):
                sps = sps2_p.tile(
                    [KT, 2 * QB], F32, tag="sps2", name=f"sps{i}{qb}{kt}"
                )
                for h in range(2):
                    r0 = h * 64
                    nc.tensor.matmul(
                        sps[:, h * QB:(h + 1) * QB],
                        krot[i][r0:r0 + 64, kt * KT:(kt + 1) * KT],
                        qrot[i][r0:r0 + 64, qsl], start=True, stop=True,
                    )
                e = e_p.tile([KT, 2 * QB], BF16, tag="e", name=f"e{i}{qb}{kt}")
                nc.scalar.activation(
                    e[:], sps[:], mybir.ActivationFunctionType.Exp
                )
                cls = schedule[(kt, qb)]
                if cls != "full":
                    for h in range(2):
                        nc.vector.tensor_mul(
                            e[:, h * QB:(h + 1) * QB],
                            e[:, h * QB:(h + 1) * QB], mts[cls][:],
                        )
                es.append(e)
                if n >= stag:
                    emit_av(n - stag, False)
            for idx in range(max(0, len(kts) - stag), len(kts)):
                emit_av(idx, idx == len(kts) - 1)

            # normalize: ctx[d,q] / sigma[q] (sigma = row 64 of cps)
            for h in range(2):
                rcp = rcp_p.tile([1, QB], F32, tag="rcp", name=f"rcp{i}{qb}{h}")
                if FASTRCP:
                    # custom-DVE op reads SBUF only: stage sigma out of PSUM
                    sg = rcp_p.tile([1, QB], F32, tag="sg", name=f"sg{i}{qb}{h}")
                    nc.vector.tensor_copy(sg[:], cps[h][64:65, :])
                    nc.vector.reciprocal_approx_fast(rcp[:], sg[:])
                else:
                    nc.vector.reciprocal(rcp[:], cps[h][64:65, :])
                rb = rb_p.tile([64, QB], F32, tag="rb", name=f"rb{i}{qb}{h}")
                nc.gpsimd.partition_broadcast(rb[:], rcp[:])
                if h == 0:
                    nc.vector.tensor_mul(
                        ctx2[i][0:64, qsl], cps[h][0:64, :], rb[:]
                    )
                else:
                    ct = ctmp_p.tile([64, QB], BF16, tag="ctmp", name=f"ct{i}{qb}")
                    nc.vector.tensor_mul(ct[:], cps[h][0:64, :], rb[:])
                    nc.sync.dma_start(ctx2[i][64:128, qsl], ct[:])

        # a2a buffers: collective m exchanges head-pair m's context.
        # chunk p = ctx2[i][:, p*SH:(p+1)*SH] -> peer p; after both
        # collectives core c holds the full 16-head context for seq slice
        # [SH*c, SH*(c+1)) of BOTH batches (group g cores hold batch g).
        a2a_in = [
            dram.tile([NCORES, 128, SH], BF16, tag=f"a2i{i}", name=f"a2a_in{i}")
            for i in range(2)
        ]
        a2a_out = [
            dram.tile([NCORES, 128, SH], BF16, tag=f"a2o{i}", name=f"a2a_out{i}")
            for i in range(2)
        ]

        def emit_a2a_stage(i, qb):
            # chunk p only needs ctx2 columns from q-block p//2: stage each
            # pair of chunks as soon as its q-block's normalize lands
            for p in (2 * qb, 2 * qb + 1):
                nc.sync.dma_start(
                    a2a_in[i][p], ctx2[i][:, p * SH:(p + 1) * SH]
                )

        def emit_a2a(i):
            nc.gpsimd.collective_compute(
                "AllToAll",
                mybir.AluOpType.bypass,
                replica_groups=[list(range(NCORES))],
                ins=[a2a_in[i].opt()],
                outs=[a2a_out[i].opt()],
            )

        # cxfb[j]: c-tile j (global heads 2j,2j+1) of the full context for
        # this core's seq slice, both batches side by side in the free dim:
        # cols 0-255 batch 0, cols 256-511 batch 1. Source: a2a collective
        # j%2 (head-pair), chunk 4b + j//2 (peer that owns batch b, pair).
        cxfb = [None] * NDT

        def emit_cxf_loads(parity):
            for j in range(parity, NDT, 2):
                t = cst.tile([128, 2 * SH], BF16, tag=f"cxf{j}", name=f"cxf{j}")
                for b in range(2):
                    nc.sync.dma_start(
                        t[:, b * SH:(b + 1) * SH],
                        a2a_out[j % 2][4 * b + j // 2],
                    )
                cxfb[j] = t

        # ---------------- emission schedule ----------------
        # All Q/K projections (and their RoPE chains) run before attention so
        # the monolithic rope DVE/GPSIMD ops never sit between attention ops
        # in the engine FIFOs (the tc semaphores are FIFO-position counters,
        # so a big op in the middle creates false dependencies).
        emit_qkproj("k", 0)
        emit_qkproj("q", 0)
        emit_qkproj("k", 1)
        emit_qkproj("q", 1)
        for st in range(4):
            emit_vproj(st)
        emit_attention(0, 0)
        emit_a2a_stage(0, 0)
        for st in range(4, 8):
            emit_vproj(st)
        emit_attention(0, 1)
        emit_a2a_stage(0, 1)
        for st in range(8, 12):
            emit_vproj(st)
        emit_attention(0, 2)
        emit_a2a_stage(0, 2)
        for st in range(12, 16):
            emit_vproj(st)
        emit_attention(0, 3)
        emit_a2a_stage(0, 3)
        emit_a2a(0)
        emit_cxf_loads(0)
        for qb in range(NQB):
            emit_attention(1, qb)
            emit_a2a_stage(1, qb)
        emit_a2a(1)

        # out-projection in two half-contractions: the pair-0 half (j even,
        # data ready after the first collective) runs on the PE while the
        # second collective is still in flight; the pair-1 half + combine
        # run after it lands.
        oes = []
        for ot in range(NDT):
            osl = slice(ot * 128, (ot + 1) * 128)
            ops = mm_p.tile([128, 2 * SH], F32, tag="mm", name=f"opse{ot}")
            for jj, j in enumerate((0, 2, 4, 6)):
                nc.tensor.matmul(
                    ops[:], wos[j][:, osl], cxfb[j][:],
                    start=(jj == 0), stop=(jj == 3),
                )
            oe = cst.tile([128, 2 * SH], F32, tag=f"oe{ot}", name=f"oe{ot}")
            nc.vector.tensor_copy(oe[:], ops[:])
            oes.append(oe)
        emit_cxf_loads(1)
        for ot in range(NDT):
            osl = slice(ot * 128, (ot + 1) * 128)
            ops = mm_p.tile([128, 2 * SH], F32, tag="mm", name=f"opso{ot}")
            for jj, j in enumerate((1, 3, 5, 7)):
                nc.tensor.matmul(
                    ops[:], wos[j][:, osl], cxfb[j][:],
                    start=(jj == 0), stop=(jj == 3),
                )
            oT = oT_p.tile([128, 2 * SH], F32, tag="oT", name=f"oT{ot}")
            nc.vector.tensor_add(oT[:], ops[:], oes[ot][:])
            nc.sync.dma_start(out_e.ap()[osl, :], oT[:])

    nc.compile()
    return nc


def _classify_mask(mask):
    """Per (kt,qb) tile classification + packed partial tiles (S^T layout)."""
    m2 = np.asarray(mask).reshape(S, S)  # [q, k] bool
    schedule = {}
    partials = []
    for kt in range(NKT):
        for qb in range(NQB):
            sub = m2[qb * QB:(qb + 1) * QB, kt * KT:(kt + 1) * KT]
            if sub.all():
                schedule[(kt, qb)] = "full"
            elif not sub.any():
                schedule[(kt, qb)] = "skip"
            else:
                schedule[(kt, qb)] = len(partials)
                partials.append(np.ascontiguousarray(sub.T).astype(BF))
    m01 = (
        np.stack(partials)
        if partials
        else np.zeros((0, KT, QB), dtype=BF)
    )
    return schedule, m01


def kernel(inputs, segment_positions, mask, W_in, W_out):
    inputs = np.asarray(inputs, dtype=np.float32)
    segment_positions = np.asarray(segment_positions, dtype=np.int32)
    W_in = np.asarray(W_in, dtype=np.float32)
    W_out = np.asarray(W_out, dtype=np.float32)

    schedule, m01 = _classify_mask(mask)
    key = tuple(sorted(schedule.items()))
    if key not in _cache:
        _cache[key] = _build(schedule, m01.shape[0])
    nc = _cache[key]

    # ---- host-side shard prep (layout/dtype only; no math beyond tables) ----
    # W_in column e maps to head e//192, role (e%192)//64 (q/k/v), dim e%64.
    Wr = W_in.reshape(D, H, 3, HD)
    half = HD // 2
    inv_freq = (1.0 / (10000.0 ** (np.arange(half, dtype=np.float32) / half)))
    wo_full = np.ascontiguousarray(W_out).astype(BF)

    in_maps = []
    for c in range(NCORES):
        b, h0 = c // 4, HPC * (c % 4)
        xTc = np.ascontiguousarray(inputs[b].T).astype(BF)
        wqc = np.ascontiguousarray(Wr[:, h0:h0 + HPC, 0, :].reshape(D, CW)).astype(BF)
        wkc = np.ascontiguousarray(Wr[:, h0:h0 + HPC, 1, :].reshape(D, CW)).astype(BF)
        wvc = np.ascontiguousarray(Wr[:, h0:h0 + HPC, 2, :].reshape(D, CW)).astype(BF)

        ang = segment_positions[b].astype(np.float32)[None, :] * inv_freq[:, None]
        c_, s_ = np.cos(ang), np.sin(ang)  # [32, S]
        C64 = np.vstack([c_, c_])
        S64 = np.vstack([-s_, s_])
        C128 = np.vstack([C64, C64]).astype(np.float32)
        S128 = np.vstack([S64, S64]).astype(np.float32)
        scale = 1.0 / np.sqrt(HD).astype(np.float32)
        im = {
            "xT": xTc, "wq": wqc, "wk": wkc, "wv": wvc, "wo": wo_full,
            "cq": (C128 * scale).astype(BF), "sq": (S128 * scale).astype(BF),
            "ck": C128.astype(BF), "sk": S128.astype(BF),
        }
        if m01.shape[0]:
            im["m01"] = m01
        in_maps.append(im)

    if SIM:
        from concourse import bass_interp

        sim = bass_interp.MultiCoreSim(nc, NCORES)
        for c in range(NCORES):
            for k, v in in_maps[c].items():
                sim.cores[c].tensor(k)[:] = v
        sim.simulate(check_with_hw=False)
        results = [
            {"out": np.asarray(sim.cores[c].mem_tensor("out"))}
            for c in range(NCORES)
        ]
        LAST["exec_time_ns"] = None
    else:
        res = run_bass_kernel_spmd(
            nc, in_maps, core_ids=list(range(NCORES)), trace=TRACE
        )
        LAST["exec_time_ns"] = res.exec_time_ns
        LAST["results"] = res
        results = res.results

    # core c returns out^T [D, 512]: cols 0-255 = batch 0 rows 256c..,
    # cols 256-511 = batch 1 rows 256c..
    out = np.empty((B, S, D), dtype=np.float32)
    for c in range(NCORES):
        r = np.asarray(results[c]["out"])
        for b in range(B):
            out[b, c * SH:(c + 1) * SH, :] = r[:, b * SH:(b + 1) * SH].T
    return out
